# revision 21
# baseline (speedup 1.0000x reference)
"""Trainium2 Bass kernel for 3-layer GraphSAGE-mean (DenseGCN variant).

Strategy (8 NeuronCores, graph/data parallel):
  - Nodes sharded by range across cores (6250/core); edges binned by dst core
    and dst 128-block.
  - Gather tables are PAIR-PACKED: table row j holds nodes (2j, 2j+1), so
    int16 gather indices (pair ids < 25000) cover all 50000 nodes with no
    half-split.  Edge slots carry parity in dstloc2 (= dst%128 + 128*parity);
    a double-wide onehot [128, 256] (is_equal vs iota2 = [0..127|128..255])
    scatters the even/odd halves of each gathered row via two matmuls into
    one accumulating PSUM bank.
  - Gathers are batched: ONE dma_gather per group of G dst-blocks (SWDGE
    fixed overhead ~1us/call).
  - T1 = [x_hi|x_lo|...] bf16 (512B rows, full f32 precision via hi+lo).
    T2 = [h1|xp|...] fp8e4, T3 = [h2|...] fp8e4 (256B rows): halves the
    inter-layer AllGather size; aggregation noise from fp8 is averaged over
    ~16 neighbors.  Root-weight (Wr) paths stay bf16 via persistent
    feature-major copies, so fp8 only touches aggregated terms.
  - Wl applied AFTER aggregation per block (node-major out), 1/deg applied
    exactly in f32, root weight (Wr) + bias via separate PSUM, ReLU on ACT.
  - Sums of xp and h1 are computed once (L2) and reused at L3.
"""

import sys

sys.path.insert(0, "/opt/trn_rl_repo")

import os

import numpy as np
import ml_dtypes

import concourse.bass as bass
import concourse.bacc as bacc
import concourse.tile as tile
from concourse import mybir
from concourse.bass_utils import run_bass_kernel_spmd

BF16 = ml_dtypes.bfloat16
FP8 = ml_dtypes.float8_e4m3

N = 50000
E = 800000
NCORES = 8
SHARD = N // NCORES          # 6250
P = 128
NBLK = (SHARD + P - 1) // P  # 49
SHARD_PAD = NBLK * P         # 6272
NPAIR = N // 2               # 25000 pair rows per table

F0 = 64
O1, O2, O3 = 64, 128, 128
FT = 256                     # table row: 256 elems (two nodes' features)
LAYERS = int(os.environ.get("KERNEL_LAYERS", "3"))
G = int(os.environ.get("KERNEL_G", "3"))   # blocks per gather group
USE_FP8 = int(os.environ.get("KERNEL_FP8", "1"))
# SWDGE descriptor ring: one gather of n rows needs ~ n*8/9+1 descs; a single
# ring entry must fit the (ucode-fixed) 1024-desc ring -> calls of <= MAXCHUNK
# chunks (MAXCHUNK*128 rows).
DMA_SCRATCH = int(os.environ.get("KERNEL_DMA_SCRATCH", "16384"))
MAXCHUNK = int(os.environ.get("KERNEL_MAXCHUNK", "8"))

dt = mybir.dt
T23_DT = dt.float8e4 if USE_FP8 else dt.bfloat16
T23_NP = FP8 if USE_FP8 else BF16


def _bf16(x):
    return np.ascontiguousarray(x.astype(BF16))


def _wrap16(idx_flat: np.ndarray) -> np.ndarray:
    """[n] -> [128, n/16]: slot i at [i%16, i//16], replicated over 8 q7 cores."""
    n = idx_flat.shape[0]
    w = idx_flat.reshape(n // 16, 16).T
    return np.ascontiguousarray(np.tile(w, (8, 1)))


def _preprocess(edge_index: np.ndarray):
    """Bin edges -> per-core pair-gather idx / dstloc2 arrays and inv-degree.

    idx layout (per core) is GROUP-major: for each group g of G dst-blocks,
    the pair indices of its blocks (t_p chunks each, wrapped) are contiguous,
    so one dma_gather per group covers all its blocks.
    """
    src = edge_index[0].astype(np.int64)
    dst = edge_index[1].astype(np.int64)
    cnt = np.bincount(dst, minlength=N)
    inv = (1.0 / np.maximum(cnt, 1)).astype(np.float32)

    core = dst // SHARD
    blk = (dst % SHARD) // P
    parity = src & 1
    # slots sorted by (core, block, src-parity): chunks are parity-pure, so
    # each chunk takes a single-wide onehot and one matmul on a static half
    # of the gathered pair row.
    key = (core * NBLK + blk) * 2 + parity
    counts = np.bincount(key, minlength=NCORES * NBLK * 2)
    t_e = int(np.ceil(counts[0::2].max() / P))
    t_o = int(np.ceil(counts[1::2].max() / P))
    t_p = t_e + t_o

    order = np.argsort(key, kind="stable")
    skey = key[order]
    ssrc = src[order]
    sdst = dst[order]
    group_start = np.searchsorted(skey, np.arange(NCORES * NBLK * 2))
    pos = np.arange(E) - group_start[skey]

    scoreb = skey // 2
    spar = skey % 2
    slot = (scoreb % NBLK) * (t_p * P) + spar * (t_e * P) + pos
    score = scoreb // NBLK

    ngrp = (NBLK + G - 1) // G
    idx_cores, dstloc_cores, inv_cores = [], [], []
    for c in range(NCORES):
        m = score == c
        sl = slot[m]
        sidx = np.zeros(NBLK * t_p * P, np.int16)
        sdl = np.full(NBLK * t_p * P, -1.0, np.float32)
        pair = ssrc[m] >> 1
        assert pair.max(initial=0) < NPAIR
        sidx[sl] = pair.astype(np.int16)
        sdl[sl] = (sdst[m] % SHARD % P).astype(np.float32)

        # wrap indices group-major
        sidx3 = sidx.reshape(NBLK, t_p * P)
        segs = []
        for g in range(ngrp):
            b0, b1 = g * G, min((g + 1) * G, NBLK)
            for b in range(b0, b1):
                segs.append(_wrap16(sidx3[b]))
        idx_cores.append(np.ascontiguousarray(np.concatenate(segs, axis=1)))

        # dstloc [128, NBLK*t_p]: [p, b*t_p+cc] = dst%128 of slot cc*128+p
        dl = sdl.reshape(NBLK * t_p, P).T
        dstloc_cores.append(np.ascontiguousarray(dl))

        iv = np.ones(SHARD_PAD, np.float32)
        iv[:SHARD] = inv[c * SHARD : (c + 1) * SHARD]
        inv_cores.append(np.ascontiguousarray(iv.reshape(NBLK, P).T))

    return idx_cores, dstloc_cores, inv_cores, t_e, t_o


def _build_nc(t_e: int, t_o: int):
    t_p = t_e + t_o
    ngrp = (NBLK + G - 1) // G
    ndesc_per_call = (MAXCHUNK * P * 8) // 9 + 1
    assert ndesc_per_call <= DMA_SCRATCH // 16, (
        f"gather of {MAXCHUNK * P} rows ({ndesc_per_call} descs) exceeds the "
        f"SWDGE ring ({DMA_SCRATCH // 16} descs)"
    )
    nc = bacc.Bacc(
        "TRN2",
        target_bir_lowering=False,
        debug=False,
        dynamic_dma_scratch_size=DMA_SCRATCH,
    )

    # ---- I/O ----
    t1_in = nc.dram_tensor("t1", [NPAIR, FT], dt.bfloat16, kind="ExternalInput")
    xt_in = nc.dram_tensor("xt", [F0, SHARD_PAD], dt.bfloat16, kind="ExternalInput")
    idx_in = nc.dram_tensor(
        "idx", [128, NBLK * t_p * 8], dt.int16, kind="ExternalInput"
    )
    dstloc_in = nc.dram_tensor(
        "dstloc", [128, NBLK * t_p], dt.float32, kind="ExternalInput"
    )
    inv_in = nc.dram_tensor("invd", [128, NBLK], dt.float32, kind="ExternalInput")
    iota_in = nc.dram_tensor("iota", [128, 128], dt.bfloat16, kind="ExternalInput")
    ident_in = nc.dram_tensor("ident", [128, 128], dt.bfloat16, kind="ExternalInput")

    wp_in = nc.dram_tensor("Wp", [F0, O1], dt.bfloat16, kind="ExternalInput")
    bp_in = nc.dram_tensor("bp", [1, O1], dt.bfloat16, kind="ExternalInput")
    w1c_in = nc.dram_tensor("W1c", [128, O1], dt.bfloat16, kind="ExternalInput")
    bl1_in = nc.dram_tensor("bl1", [1, O1], dt.bfloat16, kind="ExternalInput")
    wr1_in = nc.dram_tensor("Wr1", [F0, O1], dt.bfloat16, kind="ExternalInput")
    w2c_in = nc.dram_tensor("W2c", [128, O2], dt.bfloat16, kind="ExternalInput")
    bl2_in = nc.dram_tensor("bl2", [1, O2], dt.bfloat16, kind="ExternalInput")
    wr2a_in = nc.dram_tensor("Wr2a", [64, O2], dt.bfloat16, kind="ExternalInput")
    wr2b_in = nc.dram_tensor("Wr2b", [64, O2], dt.bfloat16, kind="ExternalInput")
    w3c_in = nc.dram_tensor("W3c", [128, O3], dt.bfloat16, kind="ExternalInput")
    wl3h2_in = nc.dram_tensor("Wl3h2", [128, O3], dt.bfloat16, kind="ExternalInput")
    bl3_in = nc.dram_tensor("bl3", [1, O3], dt.bfloat16, kind="ExternalInput")
    wr3a_in = nc.dram_tensor("Wr3a", [64, O3], dt.bfloat16, kind="ExternalInput")
    wr3b_in = nc.dram_tensor("Wr3b", [64, O3], dt.bfloat16, kind="ExternalInput")
    wr3c_in = nc.dram_tensor("Wr3c", [128, O3], dt.bfloat16, kind="ExternalInput")

    h3_out = nc.dram_tensor("h3", [SHARD, O3], dt.float32, kind="ExternalOutput")

    with tile.TileContext(nc) as tc:
        with (
            tc.tile_pool(name="cons", bufs=1) as cons,
            tc.tile_pool(name="sbuf", bufs=2) as sb,
            tc.tile_pool(name="psum", bufs=2, space="PSUM") as ps,
            tc.tile_pool(name="dram", bufs=1, space="DRAM") as dr,
        ):
            # ---- constants -> SBUF ----
            iota_t = cons.tile([128, 128], dt.bfloat16)
            nc.sync.dma_start(iota_t[:], iota_in[:])
            ident_t = cons.tile([128, 128], dt.bfloat16)
            nc.sync.dma_start(ident_t[:], ident_in[:])
            idx_t = cons.tile([128, NBLK * t_p * 8], dt.int16)
            nc.sync.dma_start(idx_t[:], idx_in[:])
            dstloc_t = cons.tile([128, NBLK * t_p], dt.float32)
            nc.sync.dma_start(dstloc_t[:], dstloc_in[:])
            inv_t = cons.tile([128, NBLK], dt.float32)
            nc.sync.dma_start(inv_t[:], inv_in[:])
            xt_t = cons.tile([F0, SHARD_PAD], dt.bfloat16)
            nc.sync.dma_start(xt_t[:], xt_in[:])
            ones_t = cons.tile([1, 128], dt.bfloat16)
            nc.gpsimd.memset(ones_t[:], 1.0)

            def load_w(name, src, shape):
                t = cons.tile(list(shape), dt.bfloat16, name=name)
                nc.sync.dma_start(t[:], src[:])
                return t

            wp_t = load_w("wp_t", wp_in, (F0, O1))
            bp_t = load_w("bp_t", bp_in, (1, O1))
            w1c_t = load_w("w1c_t", w1c_in, (128, O1))
            bl1_t = load_w("bl1_t", bl1_in, (1, O1))
            wr1_t = load_w("wr1_t", wr1_in, (F0, O1))
            w2c_t = load_w("w2c_t", w2c_in, (128, O2))
            bl2_t = load_w("bl2_t", bl2_in, (1, O2))
            wr2a_t = load_w("wr2a_t", wr2a_in, (64, O2))
            wr2b_t = load_w("wr2b_t", wr2b_in, (64, O2))
            w3c_t = load_w("w3c_t", w3c_in, (128, O3))
            wl3h2_t = load_w("wl3h2_t", wl3h2_in, (128, O3))
            bl3_t = load_w("bl3_t", bl3_in, (1, O3))
            wr3a_t = load_w("wr3a_t", wr3a_in, (64, O3))
            wr3b_t = load_w("wr3b_t", wr3b_in, (64, O3))
            wr3c_t = load_w("wr3c_t", wr3c_in, (128, O3))

            # ---- persistent feature-major activations (local shard) ----
            xpT = cons.tile([O1, SHARD_PAD], dt.bfloat16)
            h1T = cons.tile([O1, SHARD_PAD], dt.bfloat16)
            h2T = cons.tile([O2, SHARD_PAD], dt.bfloat16)
            s12T = cons.tile([128, SHARD_PAD], dt.bfloat16)  # saved [sum(h1); sum(xp)]
            xp_sb = cons.tile([128, NBLK, 64], dt.bfloat16)  # xp staging (bf16)

            # ---- DRAM tables (pair-packed rows) ----
            t2_shard = dr.tile([SHARD // 2, FT], T23_DT)
            t2_full = dr.tile([NPAIR, FT], T23_DT, addr_space="Shared")
            t3_shard = dr.tile([SHARD // 2, FT], T23_DT)
            t3_full = dr.tile([NPAIR, FT], T23_DT, addr_space="Shared")

            def transpose_to(dst_col0, src_nm, rows, name):
                """src_nm [128, rows] bf16 node-major -> dst slice [rows, 128]."""
                pt = ps.tile([rows, 128], dt.bfloat16, space="PSUM", tag="pt")
                nc.tensor.transpose(out=pt[:], in_=src_nm, identity=ident_t[:])
                nc.vector.tensor_copy(out=dst_col0, in_=pt[:])

            def gather_group(g, table, ygdt, ygtag):
                """One batched pair-gather for group g's blocks -> yg tile."""
                b0 = g * G
                nb = min(G, NBLK - b0)
                yg = sb.tile([128, G * t_p, FT], ygdt, tag=ygtag, bufs=2)
                ibase = b0 * t_p * 8
                for s0 in range(0, nb * t_p, MAXCHUNK):
                    s1 = min(s0 + MAXCHUNK, nb * t_p)
                    n_idx = (s1 - s0) * P
                    nc.gpsimd.dma_gather(
                        yg[:, s0:s1, :],
                        table,
                        idx_t[:, ibase + s0 * 8 : ibase + s1 * 8],
                        n_idx,
                        n_idx,
                        FT,
                    )
                return yg, nb

            def agg_block(yg, nb, bl, b):
                """Onehot matmuls for block b (local bl) -> psum [128f, 128dst].

                Chunks cc < t_e hold even-src slots (pair row half 0:128),
                the rest odd-src slots (half 128:256)."""
                pagg = ps.tile([128, 128], dt.float32, space="PSUM", tag="pagg")
                for cc in range(t_p):
                    pos = bl * t_p + cc
                    half = slice(0, 128) if cc < t_e else slice(128, 256)
                    oh = sb.tile([128, 128], dt.bfloat16, tag="oh", bufs=6)
                    nc.vector.tensor_scalar(
                        out=oh[:],
                        in0=iota_t[:],
                        scalar1=dstloc_t[:, b * t_p + cc : b * t_p + cc + 1],
                        scalar2=None,
                        op0=mybir.AluOpType.is_equal,
                    )
                    nc.tensor.matmul(
                        out=pagg[:],
                        lhsT=yg[:, pos, half],
                        rhs=oh[:],
                        start=(cc == 0),
                        stop=(cc == t_p - 1),
                    )
                return pagg

            def epilogue(b, pmean, pxr, out_ap):
                """out = relu(pmean * inv + pxr); writes to out_ap."""
                ocols = pmean.shape[-1]
                tmul = sb.tile([128, ocols], dt.float32, tag="tmul")
                nc.vector.tensor_scalar(
                    out=tmul[:],
                    in0=pmean[:],
                    scalar1=inv_t[:, b : b + 1],
                    scalar2=None,
                    op0=mybir.AluOpType.mult,
                )
                tadd = sb.tile([128, ocols], dt.float32, tag="tadd")
                nc.vector.tensor_tensor(
                    out=tadd[:], in0=tmul[:], in1=pxr[:], op=mybir.AluOpType.add
                )
                nc.scalar.activation(
                    out=out_ap, in_=tadd[:], func=mybir.ActivationFunctionType.Relu
                )

            # ================= Phase 1: xp per block =================
            for b in range(NBLK):
                xtb = xt_t[:, b * P : (b + 1) * P]
                pxp = ps.tile([128, O1], dt.float32, space="PSUM", tag="pmean")
                nc.tensor.matmul(
                    out=pxp[:], lhsT=xtb, rhs=wp_t[:], start=True, stop=False
                )
                nc.tensor.matmul(
                    out=pxp[:], lhsT=ones_t[:], rhs=bp_t[:], start=False, stop=True
                )
                nc.scalar.activation(
                    out=xp_sb[:, b, :],
                    in_=pxp[:],
                    func=mybir.ActivationFunctionType.Relu,
                )
                transpose_to(xpT[:, b * P : (b + 1) * P], xp_sb[:, b, :], O1, "xp")

            # ================= Phase 2: layer 1 =================
            for g in range(ngrp):
                yg, nb = gather_group(g, t1_in[:], dt.bfloat16, "yg1")
                for bl in range(nb):
                    b = g * G + bl
                    pagg = agg_block(yg, nb, bl, b)
                    # [sum(x_hi); sum(x_lo)] -> one K=128 matmul with [Wl1; Wl1]
                    sxT = sb.tile([128, 128], dt.bfloat16, tag="sxT")
                    nc.vector.tensor_copy(out=sxT[:], in_=pagg[:])
                    pmean = ps.tile([128, O1], dt.float32, space="PSUM", tag="pmean")
                    nc.tensor.matmul(
                        out=pmean[:], lhsT=sxT[:], rhs=w1c_t[:], start=True, stop=True
                    )
                    pxr = ps.tile([128, O1], dt.float32, space="PSUM", tag="pxr")
                    xtb = xt_t[:, b * P : (b + 1) * P]
                    nc.tensor.matmul(
                        out=pxr[:], lhsT=xtb, rhs=wr1_t[:], start=True, stop=False
                    )
                    nc.tensor.matmul(
                        out=pxr[:], lhsT=ones_t[:], rhs=bl1_t[:], start=False, stop=True
                    )
                    # h1 (bf16) -> rotating staging; then [h1|xp] -> T23 dtype tile
                    h1b = sb.tile([128, 64], dt.bfloat16, tag="h1b", bufs=4)
                    epilogue(b, pmean, pxr, h1b[:])
                    transpose_to(h1T[:, b * P : (b + 1) * P], h1b[:], O1, "h1")
                    st2 = sb.tile([128, 128], T23_DT, tag="st2", bufs=4)
                    nc.vector.tensor_copy(out=st2[:, 0:64], in_=h1b[:])
                    nc.vector.tensor_copy(out=st2[:, 64:128], in_=xp_sb[:, b, :])
                    # [128 nodes, 128B] bytes == [64 pair rows, 256B]
                    rows = min(P, SHARD - b * P)
                    nc.sync.dma_start(
                        t2_shard[b * 64 : b * 64 + rows // 2, :], st2[:rows, :]
                    )

            if LAYERS == 1:
                for b in range(NBLK):
                    rows = min(P, SHARD - b * P)
                    nc.gpsimd.dma_start(
                        out=h3_out[b * P : b * P + rows, 0:64],
                        in_=h1T[:, b * P : b * P + rows],
                    )

            if LAYERS >= 2:
                nc.gpsimd.collective_compute(
                    "AllGather",
                    mybir.AluOpType.bypass,
                    replica_groups=[list(range(NCORES))],
                    ins=[t2_shard[:]],
                    outs=[t2_full[:]],
                )

                # ================= Phase 3: layer 2 =================
                for g in range(ngrp):
                    yg, nb = gather_group(g, t2_full[:], T23_DT, "yg")
                    for bl in range(nb):
                        b = g * G + bl
                        pagg = agg_block(yg, nb, bl, b)
                        # rows 0:64 = sum(h1), 64:128 = sum(xp); save whole block
                        nc.vector.tensor_copy(
                            out=s12T[:, b * P : (b + 1) * P], in_=pagg[:]
                        )
                        pmean = ps.tile([128, O2], dt.float32, space="PSUM", tag="pmean")
                        nc.tensor.matmul(
                            out=pmean[:],
                            lhsT=s12T[:, b * P : (b + 1) * P],
                            rhs=w2c_t[:],
                            start=True,
                            stop=True,
                        )
                        pxr = ps.tile([128, O2], dt.float32, space="PSUM", tag="pxr")
                        nc.tensor.matmul(
                            out=pxr[:],
                            lhsT=xpT[:, b * P : (b + 1) * P],
                            rhs=wr2a_t[:],
                            start=True,
                            stop=False,
                        )
                        nc.tensor.matmul(
                            out=pxr[:],
                            lhsT=h1T[:, b * P : (b + 1) * P],
                            rhs=wr2b_t[:],
                            start=False,
                            stop=False,
                        )
                        nc.tensor.matmul(
                            out=pxr[:], lhsT=ones_t[:], rhs=bl2_t[:],
                            start=False, stop=True,
                        )
                        h2b = sb.tile([128, 128], dt.bfloat16, tag="h2b", bufs=4)
                        epilogue(b, pmean, pxr, h2b[:])
                        transpose_to(h2T[:, b * P : (b + 1) * P], h2b[:], O2, "h2")
                        st3 = sb.tile([128, 128], T23_DT, tag="st2", bufs=4)
                        nc.vector.tensor_copy(out=st3[:], in_=h2b[:])
                        rows = min(P, SHARD - b * P)
                        nc.sync.dma_start(
                            t3_shard[b * 64 : b * 64 + rows // 2, :], st3[:rows, :]
                        )

            if LAYERS == 2:
                for b in range(NBLK):
                    rows = min(P, SHARD - b * P)
                    nc.gpsimd.dma_start(
                        out=h3_out[b * P : b * P + rows, :],
                        in_=h2T[:, b * P : b * P + rows],
                    )

            if LAYERS >= 3:
                nc.gpsimd.collective_compute(
                    "AllGather",
                    mybir.AluOpType.bypass,
                    replica_groups=[list(range(NCORES))],
                    ins=[t3_shard[:]],
                    outs=[t3_full[:]],
                )

                # ================= Phase 4: layer 3 =================
                for g in range(ngrp):
                    yg, nb = gather_group(g, t3_full[:], T23_DT, "yg")
                    for bl in range(nb):
                        b = g * G + bl
                        pagg = agg_block(yg, nb, bl, b)
                        sh2T = sb.tile([128, 128], dt.bfloat16, tag="sh2T")
                        nc.vector.tensor_copy(out=sh2T[:], in_=pagg[:])
                        pmean = ps.tile([128, O3], dt.float32, space="PSUM", tag="pmean")
                        nc.tensor.matmul(
                            out=pmean[:],
                            lhsT=s12T[:, b * P : (b + 1) * P],
                            rhs=w3c_t[:],
                            start=True,
                            stop=False,
                        )
                        nc.tensor.matmul(
                            out=pmean[:],
                            lhsT=sh2T[:],
                            rhs=wl3h2_t[:],
                            start=False,
                            stop=True,
                        )
                        pxr = ps.tile([128, O3], dt.float32, space="PSUM", tag="pxr")
                        nc.tensor.matmul(
                            out=pxr[:],
                            lhsT=xpT[:, b * P : (b + 1) * P],
                            rhs=wr3a_t[:],
                            start=True,
                            stop=False,
                        )
                        nc.tensor.matmul(
                            out=pxr[:],
                            lhsT=h1T[:, b * P : (b + 1) * P],
                            rhs=wr3b_t[:],
                            start=False,
                            stop=False,
                        )
                        nc.tensor.matmul(
                            out=pxr[:],
                            lhsT=h2T[:, b * P : (b + 1) * P],
                            rhs=wr3c_t[:],
                            start=False,
                            stop=False,
                        )
                        nc.tensor.matmul(
                            out=pxr[:], lhsT=ones_t[:], rhs=bl3_t[:],
                            start=False, stop=True,
                        )
                        h3t = sb.tile([128, O3], dt.float32, tag="h3t")
                        epilogue(b, pmean, pxr, h3t[:])
                        rows = min(P, SHARD - b * P)
                        nc.sync.dma_start(
                            h3_out[b * P : b * P + rows, :], h3t[:rows, :]
                        )

    nc.compile()
    return nc


_NC_CACHE: dict = {}


def _make_in_maps(inputs, idx_cores, dstloc_cores, inv_cores):
    x = np.asarray(inputs["x"], np.float32)
    # pair-packed T1 row j = [x_hi(2j) | x_lo(2j) | x_hi(2j+1) | x_lo(2j+1)]
    x_hi = x.astype(BF16)
    x_lo = (x - x_hi.astype(np.float32)).astype(BF16)
    t1 = np.concatenate([x_hi, x_lo], axis=1).reshape(NPAIR, FT)
    t1 = np.ascontiguousarray(t1)

    iota = np.broadcast_to(np.arange(128, dtype=np.float32), (128, 128))
    iota = np.ascontiguousarray(iota)
    ident = np.eye(128, dtype=np.float32)

    def wrow(v):
        return np.ascontiguousarray(v.reshape(1, -1))

    common = dict(
        t1=t1,
        iota=_bf16(iota),
        ident=_bf16(ident),
        Wp=_bf16(inputs["Wp"]),
        bp=_bf16(wrow(inputs["bp"])),
        # table col order L1: [x_hi | x_lo] -> both multiply Wl1
        W1c=_bf16(np.vstack([inputs["Wl1"], inputs["Wl1"]])),
        bl1=_bf16(wrow(inputs["bl1"])),
        Wr1=_bf16(inputs["Wr1"]),
        # table col order L2: [h1 | xp] -> [Wl2[64:], Wl2[:64]]
        W2c=_bf16(np.vstack([inputs["Wl2"][64:128], inputs["Wl2"][0:64]])),
        bl2=_bf16(wrow(inputs["bl2"])),
        Wr2a=_bf16(inputs["Wr2"][0:64]),
        Wr2b=_bf16(inputs["Wr2"][64:128]),
        # s12T rows: [sum(h1); sum(xp)] -> [Wl3[64:128], Wl3[0:64]]
        W3c=_bf16(np.vstack([inputs["Wl3"][64:128], inputs["Wl3"][0:64]])),
        Wl3h2=_bf16(inputs["Wl3"][128:256]),
        bl3=_bf16(wrow(inputs["bl3"])),
        Wr3a=_bf16(inputs["Wr3"][0:64]),
        Wr3b=_bf16(inputs["Wr3"][64:128]),
        Wr3c=_bf16(inputs["Wr3"][128:256]),
    )

    in_maps = []
    for c in range(NCORES):
        xs = np.zeros((F0, SHARD_PAD), np.float32)
        xs[:, :SHARD] = x[c * SHARD : (c + 1) * SHARD].T
        in_maps.append(
            dict(
                common,
                xt=_bf16(xs),
                idx=idx_cores[c],
                dstloc=dstloc_cores[c],
                invd=inv_cores[c],
            )
        )
    return in_maps


def kernel(**inputs: np.ndarray) -> np.ndarray:
    edge_index = np.asarray(inputs["edge_index"])
    idx_cores, dstloc_cores, inv_cores, t_e, t_o = _preprocess(edge_index)

    ck = (t_e, t_o)
    if ck not in _NC_CACHE:
        _NC_CACHE[ck] = _build_nc(t_e, t_o)
    nc = _NC_CACHE[ck]

    in_maps = _make_in_maps(inputs, idx_cores, dstloc_cores, inv_cores)
    res = run_bass_kernel_spmd(nc, in_maps, core_ids=list(range(NCORES)))
    out = np.concatenate([res.results[c]["h3"] for c in range(NCORES)], axis=0)
    return out.astype(np.float32)


# revision 36
# speedup vs baseline: 1.2586x; 1.2586x over previous
"""Trainium2 Bass kernel for 3-layer GraphSAGE-mean (DenseGCN variant).

Strategy (8 NeuronCores, graph/data parallel):
  - Nodes sharded by range across cores (6250/core); edges binned by dst core
    and dst 128-block.
  - Gather tables are PAIR-PACKED: table row j holds nodes (2j, 2j+1), so
    int16 gather indices (pair ids < 25000) cover all 50000 nodes with no
    half-split.  Edge slots carry parity in dstloc2 (= dst%128 + 128*parity);
    a double-wide onehot [128, 256] (is_equal vs iota2 = [0..127|128..255])
    scatters the even/odd halves of each gathered row via two matmuls into
    one accumulating PSUM bank.
  - Gathers are batched: ONE dma_gather per group of G dst-blocks (SWDGE
    fixed overhead ~1us/call).
  - T1 = [x_hi|x_lo|...] bf16 (512B rows, full f32 precision via hi+lo).
    T2 = [h1|xp|...] fp8e4, T3 = [h2|...] fp8e4 (256B rows): halves the
    inter-layer AllGather size; aggregation noise from fp8 is averaged over
    ~16 neighbors.  Root-weight (Wr) paths stay bf16 via persistent
    feature-major copies, so fp8 only touches aggregated terms.
  - Wl applied AFTER aggregation per block (node-major out), 1/deg applied
    exactly in f32, root weight (Wr) + bias via separate PSUM, ReLU on ACT.
  - Sums of xp and h1 are computed once (L2) and reused at L3.
"""

import sys

sys.path.insert(0, "/opt/trn_rl_repo")

import os

import numpy as np
import ml_dtypes

import concourse.bass as bass
import concourse.bacc as bacc
import concourse.tile as tile
from concourse import mybir
from concourse.bass_utils import run_bass_kernel_spmd

BF16 = ml_dtypes.bfloat16
FP8 = ml_dtypes.float8_e4m3

N = 50000
E = 800000
NCORES = 8
SHARD = N // NCORES          # 6250
P = 128
NBLK = (SHARD + P - 1) // P  # 49
SHARD_PAD = NBLK * P         # 6272
NPAIR = N // 2               # 25000 pair rows per table

F0 = 64
O1, O2, O3 = 64, 128, 128
FT = 256                     # table row: 256 elems (two nodes' features)
LAYERS = int(os.environ.get("KERNEL_LAYERS", "3"))
G = int(os.environ.get("KERNEL_G", "3"))   # blocks per gather group
USE_FP8 = int(os.environ.get("KERNEL_FP8", "1"))
# SWDGE descriptor ring: one gather of n rows needs ~ n*8/9+1 descs; a single
# ring entry must fit the (ucode-fixed) 1024-desc ring -> calls of <= MAXCHUNK
# chunks (MAXCHUNK*128 rows).
DMA_SCRATCH = int(os.environ.get("KERNEL_DMA_SCRATCH", "16384"))
MAXCHUNK = int(os.environ.get("KERNEL_MAXCHUNK", "8"))
# The inter-layer AllGathers are split into chunk collectives over dst-block
# ranges so each chunk starts as soon as its staging sub-range is written,
# overlapping the collective with the producing layer's tail.  Table rows are
# permuted so each chunk's AllGather output is contiguous.
SPLITS = [int(x) for x in os.environ.get("KERNEL_SPLITS", "9,12,12,16").split(",")]
assert sum(SPLITS) == NBLK
_SPLIT_META = []  # (node0, node1, pairs_per_core, table_base_row)
_n0, _base = 0, 0
for _cnt in SPLITS:
    _n1 = min(_n0 + _cnt * P, SHARD)
    _pairs = (_n1 - _n0) // 2
    _SPLIT_META.append((_n0, _n1, _pairs, _base))
    _base += NCORES * _pairs
    _n0 = _n1
assert _n0 == SHARD and _base == NPAIR


def _table_row(g: np.ndarray) -> np.ndarray:
    """Global node id -> pair-table row under the split-permuted layout."""
    c = g // SHARD
    r = g % SHARD
    row = np.zeros_like(g)
    for a0, a1, pairs, base in _SPLIT_META:
        msk = (r >= a0) & (r < a1)
        row[msk] = base + c[msk] * pairs + ((r[msk] - a0) >> 1)
    return row

dt = mybir.dt
T23_DT = dt.float8e4 if USE_FP8 else dt.bfloat16
T23_NP = FP8 if USE_FP8 else BF16


def _bf16(x):
    return np.ascontiguousarray(x.astype(BF16))


def _wrap16(idx_flat: np.ndarray) -> np.ndarray:
    """[n] -> [128, n/16]: slot i at [i%16, i//16], replicated over 8 q7 cores."""
    n = idx_flat.shape[0]
    w = idx_flat.reshape(n // 16, 16).T
    return np.ascontiguousarray(np.tile(w, (8, 1)))


def _preprocess(edge_index: np.ndarray):
    """Bin edges -> per-core pair-gather idx / dstloc2 arrays and inv-degree.

    idx layout (per core) is GROUP-major: for each group g of G dst-blocks,
    the pair indices of its blocks (t_p chunks each, wrapped) are contiguous,
    so one dma_gather per group covers all its blocks.
    """
    src = edge_index[0].astype(np.int64)
    dst = edge_index[1].astype(np.int64)
    cnt = np.bincount(dst, minlength=N)
    inv = (1.0 / np.maximum(cnt, 1)).astype(np.float32)

    core = dst // SHARD
    blk = (dst % SHARD) // P
    parity = src & 1
    # slots sorted by (core, block, src-parity): chunks are parity-pure, so
    # each chunk takes a single-wide onehot and one matmul on a static half
    # of the gathered pair row.  Chunk budgets are PER BLOCK (max over cores),
    # not global, so lightly-loaded blocks aren't padded to the global worst.
    key = (core * NBLK + blk) * 2 + parity
    counts2 = np.bincount(key, minlength=NCORES * NBLK * 2).reshape(
        NCORES, NBLK, 2
    )
    t_e_arr = tuple(
        int(v) for v in np.ceil(counts2[:, :, 0].max(axis=0) / P).astype(int)
    )
    t_o_arr = tuple(
        int(v) for v in np.ceil(counts2[:, :, 1].max(axis=0) / P).astype(int)
    )
    t_b = np.array(t_e_arr) + np.array(t_o_arr)
    off_b = np.concatenate([[0], np.cumsum(t_b)])  # chunk offsets, [NBLK+1]
    tot = int(off_b[-1])

    order = np.argsort(key, kind="stable")
    skey = key[order]
    ssrc = src[order]
    sdst = dst[order]
    group_start = np.searchsorted(skey, np.arange(NCORES * NBLK * 2))
    pos = np.arange(E) - group_start[skey]

    scoreb = skey // 2
    spar = skey % 2
    b_of = scoreb % NBLK
    slot = (off_b[b_of] + spar * np.array(t_e_arr)[b_of]) * P + pos
    score = scoreb // NBLK

    ngrp = (NBLK + G - 1) // G
    idx_cores, dstloc_cores, inv_cores = [], [], []
    for c in range(NCORES):
        m = score == c
        sl = slot[m]
        sidx = np.zeros(tot * P, np.int16)
        sdl = np.full(tot * P, -1.0, np.float32)
        pair = _table_row(ssrc[m])
        assert pair.max(initial=0) < NPAIR
        sidx[sl] = pair.astype(np.int16)
        sdl[sl] = (sdst[m] % SHARD % P).astype(np.float32)

        # wrap indices group-major (ragged per block)
        segs = []
        for g in range(ngrp):
            b0, b1 = g * G, min((g + 1) * G, NBLK)
            for b in range(b0, b1):
                segs.append(_wrap16(sidx[off_b[b] * P : off_b[b + 1] * P]))
        idx_cores.append(np.ascontiguousarray(np.concatenate(segs, axis=1)))

        # dstloc [128, tot]: [p, off_b[b]+cc] = dst%128 of that chunk's slot p
        dl = sdl.reshape(tot, P).T
        dstloc_cores.append(np.ascontiguousarray(dl))

        iv = np.ones(SHARD_PAD, np.float32)
        iv[:SHARD] = inv[c * SHARD : (c + 1) * SHARD]
        inv_cores.append(np.ascontiguousarray(iv.reshape(NBLK, P).T))

    return idx_cores, dstloc_cores, inv_cores, t_e_arr, t_o_arr


def _build_nc(t_e_arr: tuple, t_o_arr: tuple):
    t_b = np.array(t_e_arr) + np.array(t_o_arr)
    off_b = np.concatenate([[0], np.cumsum(t_b)])
    tot = int(off_b[-1])
    ngrp = (NBLK + G - 1) // G
    gmax = max(
        int(off_b[min((g + 1) * G, NBLK)] - off_b[g * G]) for g in range(ngrp)
    )
    ndesc_per_call = (MAXCHUNK * P * 8) // 9 + 1
    assert ndesc_per_call <= DMA_SCRATCH // 16, (
        f"gather of {MAXCHUNK * P} rows ({ndesc_per_call} descs) exceeds the "
        f"SWDGE ring ({DMA_SCRATCH // 16} descs)"
    )
    nc = bacc.Bacc(
        "TRN2",
        target_bir_lowering=False,
        debug=False,
        dynamic_dma_scratch_size=DMA_SCRATCH,
    )

    # ---- I/O ----
    t1_in = nc.dram_tensor("t1", [NPAIR, FT], dt.bfloat16, kind="ExternalInput")
    xt_in = nc.dram_tensor("xt", [F0, SHARD_PAD], dt.bfloat16, kind="ExternalInput")
    idx_in = nc.dram_tensor(
        "idx", [128, tot * 8], dt.int16, kind="ExternalInput"
    )
    dstloc_in = nc.dram_tensor(
        "dstloc", [128, tot], dt.float32, kind="ExternalInput"
    )
    inv_in = nc.dram_tensor("invd", [128, NBLK], dt.float32, kind="ExternalInput")
    iota_in = nc.dram_tensor("iota", [128, 128], dt.bfloat16, kind="ExternalInput")
    ident_in = nc.dram_tensor("ident", [128, 128], dt.bfloat16, kind="ExternalInput")

    wp_in = nc.dram_tensor("Wp", [F0, O1], dt.bfloat16, kind="ExternalInput")
    bp_in = nc.dram_tensor("bp", [1, O1], dt.bfloat16, kind="ExternalInput")
    w1c_in = nc.dram_tensor("W1c", [128, O1], dt.bfloat16, kind="ExternalInput")
    bl1_in = nc.dram_tensor("bl1", [1, O1], dt.bfloat16, kind="ExternalInput")
    wr1_in = nc.dram_tensor("Wr1", [F0, O1], dt.bfloat16, kind="ExternalInput")
    w2c_in = nc.dram_tensor("W2c", [128, O2], dt.bfloat16, kind="ExternalInput")
    bl2_in = nc.dram_tensor("bl2", [1, O2], dt.bfloat16, kind="ExternalInput")
    wr2a_in = nc.dram_tensor("Wr2a", [64, O2], dt.bfloat16, kind="ExternalInput")
    wr2b_in = nc.dram_tensor("Wr2b", [64, O2], dt.bfloat16, kind="ExternalInput")
    w3c_in = nc.dram_tensor("W3c", [128, O3], dt.bfloat16, kind="ExternalInput")
    wl3h2_in = nc.dram_tensor("Wl3h2", [128, O3], dt.bfloat16, kind="ExternalInput")
    bl3_in = nc.dram_tensor("bl3", [1, O3], dt.bfloat16, kind="ExternalInput")
    wr3a_in = nc.dram_tensor("Wr3a", [64, O3], dt.bfloat16, kind="ExternalInput")
    wr3b_in = nc.dram_tensor("Wr3b", [64, O3], dt.bfloat16, kind="ExternalInput")
    wr3c_in = nc.dram_tensor("Wr3c", [128, O3], dt.bfloat16, kind="ExternalInput")

    h3_out = nc.dram_tensor("h3", [SHARD, O3], dt.float32, kind="ExternalOutput")

    with tile.TileContext(nc) as tc:
        with (
            tc.tile_pool(name="cons", bufs=1) as cons,
            tc.tile_pool(name="sbuf", bufs=2) as sb,
            tc.tile_pool(name="psum", bufs=2, space="PSUM") as ps,
            tc.tile_pool(name="dram", bufs=1, space="DRAM") as dr,
        ):
            # ---- constants -> SBUF ----
            iota_t = cons.tile([128, 128], dt.bfloat16)
            nc.sync.dma_start(iota_t[:], iota_in[:])
            ident_t = cons.tile([128, 128], dt.bfloat16)
            nc.sync.dma_start(ident_t[:], ident_in[:])
            idx_t = cons.tile([128, tot * 8], dt.int16)
            nc.sync.dma_start(idx_t[:], idx_in[:])
            dstloc_t = cons.tile([128, tot], dt.float32)
            nc.sync.dma_start(dstloc_t[:], dstloc_in[:])
            inv_t = cons.tile([128, NBLK], dt.float32)
            nc.sync.dma_start(inv_t[:], inv_in[:])
            xt_t = cons.tile([F0, SHARD_PAD], dt.bfloat16)
            nc.sync.dma_start(xt_t[:], xt_in[:])
            ones_t = cons.tile([1, 128], dt.bfloat16)
            nc.gpsimd.memset(ones_t[:], 1.0)

            def load_w(name, src, shape):
                t = cons.tile(list(shape), dt.bfloat16, name=name)
                nc.sync.dma_start(t[:], src[:])
                return t

            wp_t = load_w("wp_t", wp_in, (F0, O1))
            bp_t = load_w("bp_t", bp_in, (1, O1))
            w1c_t = load_w("w1c_t", w1c_in, (128, O1))
            bl1_t = load_w("bl1_t", bl1_in, (1, O1))
            wr1_t = load_w("wr1_t", wr1_in, (F0, O1))
            w2c_t = load_w("w2c_t", w2c_in, (128, O2))
            bl2_t = load_w("bl2_t", bl2_in, (1, O2))
            wr2a_t = load_w("wr2a_t", wr2a_in, (64, O2))
            wr2b_t = load_w("wr2b_t", wr2b_in, (64, O2))
            w3c_t = load_w("w3c_t", w3c_in, (128, O3))
            wl3h2_t = load_w("wl3h2_t", wl3h2_in, (128, O3))
            bl3_t = load_w("bl3_t", bl3_in, (1, O3))
            wr3a_t = load_w("wr3a_t", wr3a_in, (64, O3))
            wr3b_t = load_w("wr3b_t", wr3b_in, (64, O3))
            wr3c_t = load_w("wr3c_t", wr3c_in, (128, O3))

            # ---- persistent feature-major activations (local shard) ----
            xpT = cons.tile([O1, SHARD_PAD], dt.bfloat16)
            h1T = cons.tile([O1, SHARD_PAD], dt.bfloat16)
            h2T = cons.tile([O2, SHARD_PAD], dt.bfloat16)
            s12T = cons.tile([128, SHARD_PAD], dt.bfloat16)  # saved [sum(h1); sum(xp)]
            xp_sb = cons.tile([128, NBLK, 64], dt.bfloat16)  # xp staging (bf16)

            # ---- DRAM tables (pair-packed rows, split-permuted layout) ----
            t2_shards = [
                dr.tile([m[2], FT], T23_DT, name=f"t2_shard{s}")
                for s, m in enumerate(_SPLIT_META)
            ]
            t2_full = dr.tile([NPAIR, FT], T23_DT)
            t3_shards = [
                dr.tile([m[2], FT], T23_DT, name=f"t3_shard{s}")
                for s, m in enumerate(_SPLIT_META)
            ]
            t3_full = dr.tile([NPAIR, FT], T23_DT)

            def stage_block(shards, b, st, rows):
                """Write block b's [rows,128] staging to its split shard."""
                s = 0
                while b * P >= _SPLIT_META[s][1]:
                    s += 1
                off = b * 64 - _SPLIT_META[s][0] // 2
                nc.sync.dma_start(
                    shards[s][off : off + rows // 2, :], st[:rows, :]
                )

            # group index after which split s's staging is complete
            _emit_after = {}
            for s, (a0, a1, pairs, base) in enumerate(_SPLIT_META):
                last_blk = (a1 + P - 1) // P - 1
                _emit_after.setdefault(last_blk // G, []).append(s)

            def allgather_chunk(shards, full, s):
                a0, a1, pairs, base = _SPLIT_META[s]
                nc.gpsimd.collective_compute(
                    "AllGather",
                    mybir.AluOpType.bypass,
                    replica_groups=[list(range(NCORES))],
                    ins=[shards[s][:]],
                    outs=[full[base : base + NCORES * pairs, :]],
                )

            def transpose_to(dst_col0, src_nm, rows, name):
                """src_nm [128, rows] bf16 node-major -> dst slice [rows, 128]."""
                pt = ps.tile([rows, 128], dt.bfloat16, space="PSUM", tag="pt")
                nc.tensor.transpose(out=pt[:], in_=src_nm, identity=ident_t[:])
                nc.vector.tensor_copy(out=dst_col0, in_=pt[:])

            def gather_group(g, table, ygdt, ygtag):
                """One batched pair-gather for group g's blocks -> yg tile."""
                b0 = g * G
                nb = min(G, NBLK - b0)
                goff = int(off_b[b0])
                gch = int(off_b[b0 + nb]) - goff
                yg = sb.tile([128, gmax, FT], ygdt, tag=ygtag, bufs=2)
                ibase = goff * 8
                for s0 in range(0, gch, MAXCHUNK):
                    s1 = min(s0 + MAXCHUNK, gch)
                    n_idx = (s1 - s0) * P
                    nc.gpsimd.dma_gather(
                        yg[:, s0:s1, :],
                        table,
                        idx_t[:, ibase + s0 * 8 : ibase + s1 * 8],
                        n_idx,
                        n_idx,
                        FT,
                    )
                return yg, nb

            def agg_block(yg, nb, bl, b):
                """Onehot matmuls for block b (local bl) -> psum [128f, 128dst].

                Chunks cc < t_e_arr[b] hold even-src slots (pair row half
                0:128), the rest odd-src slots (half 128:256)."""
                pagg = ps.tile([128, 128], dt.float32, space="PSUM", tag="pagg")
                boff = int(off_b[b])
                goff = int(off_b[b - bl])
                tp_b = int(t_b[b])
                for cc in range(tp_b):
                    pos = boff - goff + cc
                    half = slice(0, 128) if cc < t_e_arr[b] else slice(128, 256)
                    oh = sb.tile([128, 128], dt.bfloat16, tag="oh", bufs=6)
                    nc.vector.tensor_scalar(
                        out=oh[:],
                        in0=iota_t[:],
                        scalar1=dstloc_t[:, boff + cc : boff + cc + 1],
                        scalar2=None,
                        op0=mybir.AluOpType.is_equal,
                    )
                    nc.tensor.matmul(
                        out=pagg[:],
                        lhsT=yg[:, pos, half],
                        rhs=oh[:],
                        start=(cc == 0),
                        stop=(cc == tp_b - 1),
                    )
                return pagg

            def epilogue(b, pmean, pxr, out_ap):
                """out = relu(pmean * inv + pxr); writes to out_ap."""
                ocols = pmean.shape[-1]
                tmul = sb.tile([128, ocols], dt.float32, tag="tmul")
                nc.vector.tensor_scalar(
                    out=tmul[:],
                    in0=pmean[:],
                    scalar1=inv_t[:, b : b + 1],
                    scalar2=None,
                    op0=mybir.AluOpType.mult,
                )
                tadd = sb.tile([128, ocols], dt.float32, tag="tadd")
                nc.vector.tensor_tensor(
                    out=tadd[:], in0=tmul[:], in1=pxr[:], op=mybir.AluOpType.add
                )
                nc.scalar.activation(
                    out=out_ap, in_=tadd[:], func=mybir.ActivationFunctionType.Relu
                )

            # ================= Phase 1: xp per block =================
            for b in range(NBLK):
                xtb = xt_t[:, b * P : (b + 1) * P]
                pxp = ps.tile([128, O1], dt.float32, space="PSUM", tag="pmean")
                nc.tensor.matmul(
                    out=pxp[:], lhsT=xtb, rhs=wp_t[:], start=True, stop=False
                )
                nc.tensor.matmul(
                    out=pxp[:], lhsT=ones_t[:], rhs=bp_t[:], start=False, stop=True
                )
                nc.scalar.activation(
                    out=xp_sb[:, b, :],
                    in_=pxp[:],
                    func=mybir.ActivationFunctionType.Relu,
                )
                transpose_to(xpT[:, b * P : (b + 1) * P], xp_sb[:, b, :], O1, "xp")

            # ================= Phase 2: layer 1 =================
            for g in range(ngrp):
                yg, nb = gather_group(g, t1_in[:], dt.bfloat16, "yg1")
                for bl in range(nb):
                    b = g * G + bl
                    pagg = agg_block(yg, nb, bl, b)
                    # [sum(x_hi); sum(x_lo)] -> one K=128 matmul with [Wl1; Wl1]
                    sxT = sb.tile([128, 128], dt.bfloat16, tag="sxT")
                    nc.vector.tensor_copy(out=sxT[:], in_=pagg[:])
                    pmean = ps.tile([128, O1], dt.float32, space="PSUM", tag="pmean")
                    nc.tensor.matmul(
                        out=pmean[:], lhsT=sxT[:], rhs=w1c_t[:], start=True, stop=True
                    )
                    pxr = ps.tile([128, O1], dt.float32, space="PSUM", tag="pxr")
                    xtb = xt_t[:, b * P : (b + 1) * P]
                    nc.tensor.matmul(
                        out=pxr[:], lhsT=xtb, rhs=wr1_t[:], start=True, stop=False
                    )
                    nc.tensor.matmul(
                        out=pxr[:], lhsT=ones_t[:], rhs=bl1_t[:], start=False, stop=True
                    )
                    # h1 (bf16) -> rotating staging; then [h1|xp] -> T23 dtype tile
                    h1b = sb.tile([128, 64], dt.bfloat16, tag="h1b", bufs=4)
                    epilogue(b, pmean, pxr, h1b[:])
                    transpose_to(h1T[:, b * P : (b + 1) * P], h1b[:], O1, "h1")
                    st2 = sb.tile([128, 128], T23_DT, tag="st2", bufs=4)
                    nc.vector.tensor_copy(out=st2[:, 0:64], in_=h1b[:])
                    nc.vector.tensor_copy(out=st2[:, 64:128], in_=xp_sb[:, b, :])
                    # [128 nodes, 128B] bytes == [64 pair rows, 256B]
                    rows = min(P, SHARD - b * P)
                    stage_block(t2_shards, b, st2, rows)
                if LAYERS >= 2:
                    for s_ in _emit_after.get(g, []):
                        allgather_chunk(t2_shards, t2_full, s_)

            if LAYERS == 1:
                for b in range(NBLK):
                    rows = min(P, SHARD - b * P)
                    nc.gpsimd.dma_start(
                        out=h3_out[b * P : b * P + rows, 0:64],
                        in_=h1T[:, b * P : b * P + rows],
                    )

            if LAYERS >= 2:
                # ================= Phase 3: layer 2 =================
                for g in range(ngrp):
                    yg, nb = gather_group(g, t2_full[:], T23_DT, "yg")
                    for bl in range(nb):
                        b = g * G + bl
                        pagg = agg_block(yg, nb, bl, b)
                        # rows 0:64 = sum(h1), 64:128 = sum(xp); save whole block
                        nc.vector.tensor_copy(
                            out=s12T[:, b * P : (b + 1) * P], in_=pagg[:]
                        )
                        pmean = ps.tile([128, O2], dt.float32, space="PSUM", tag="pmean")
                        nc.tensor.matmul(
                            out=pmean[:],
                            lhsT=s12T[:, b * P : (b + 1) * P],
                            rhs=w2c_t[:],
                            start=True,
                            stop=True,
                        )
                        pxr = ps.tile([128, O2], dt.float32, space="PSUM", tag="pxr")
                        nc.tensor.matmul(
                            out=pxr[:],
                            lhsT=xpT[:, b * P : (b + 1) * P],
                            rhs=wr2a_t[:],
                            start=True,
                            stop=False,
                        )
                        nc.tensor.matmul(
                            out=pxr[:],
                            lhsT=h1T[:, b * P : (b + 1) * P],
                            rhs=wr2b_t[:],
                            start=False,
                            stop=False,
                        )
                        nc.tensor.matmul(
                            out=pxr[:], lhsT=ones_t[:], rhs=bl2_t[:],
                            start=False, stop=True,
                        )
                        h2b = sb.tile([128, 128], dt.bfloat16, tag="h2b", bufs=4)
                        epilogue(b, pmean, pxr, h2b[:])
                        transpose_to(h2T[:, b * P : (b + 1) * P], h2b[:], O2, "h2")
                        st3 = sb.tile([128, 128], T23_DT, tag="st2", bufs=4)
                        nc.vector.tensor_copy(out=st3[:], in_=h2b[:])
                        rows = min(P, SHARD - b * P)
                        stage_block(t3_shards, b, st3, rows)
                    if LAYERS >= 3:
                        for s_ in _emit_after.get(g, []):
                            allgather_chunk(t3_shards, t3_full, s_)

            if LAYERS == 2:
                for b in range(NBLK):
                    rows = min(P, SHARD - b * P)
                    nc.gpsimd.dma_start(
                        out=h3_out[b * P : b * P + rows, :],
                        in_=h2T[:, b * P : b * P + rows],
                    )

            if LAYERS >= 3:
                # ================= Phase 4: layer 3 =================
                for g in range(ngrp):
                    yg, nb = gather_group(g, t3_full[:], T23_DT, "yg")
                    for bl in range(nb):
                        b = g * G + bl
                        pagg = agg_block(yg, nb, bl, b)
                        sh2T = sb.tile([128, 128], dt.bfloat16, tag="sh2T")
                        nc.vector.tensor_copy(out=sh2T[:], in_=pagg[:])
                        pmean = ps.tile([128, O3], dt.float32, space="PSUM", tag="pmean")
                        nc.tensor.matmul(
                            out=pmean[:],
                            lhsT=s12T[:, b * P : (b + 1) * P],
                            rhs=w3c_t[:],
                            start=True,
                            stop=False,
                        )
                        nc.tensor.matmul(
                            out=pmean[:],
                            lhsT=sh2T[:],
                            rhs=wl3h2_t[:],
                            start=False,
                            stop=True,
                        )
                        pxr = ps.tile([128, O3], dt.float32, space="PSUM", tag="pxr")
                        nc.tensor.matmul(
                            out=pxr[:],
                            lhsT=xpT[:, b * P : (b + 1) * P],
                            rhs=wr3a_t[:],
                            start=True,
                            stop=False,
                        )
                        nc.tensor.matmul(
                            out=pxr[:],
                            lhsT=h1T[:, b * P : (b + 1) * P],
                            rhs=wr3b_t[:],
                            start=False,
                            stop=False,
                        )
                        nc.tensor.matmul(
                            out=pxr[:],
                            lhsT=h2T[:, b * P : (b + 1) * P],
                            rhs=wr3c_t[:],
                            start=False,
                            stop=False,
                        )
                        nc.tensor.matmul(
                            out=pxr[:], lhsT=ones_t[:], rhs=bl3_t[:],
                            start=False, stop=True,
                        )
                        h3t = sb.tile([128, O3], dt.float32, tag="h3t")
                        epilogue(b, pmean, pxr, h3t[:])
                        rows = min(P, SHARD - b * P)
                        nc.sync.dma_start(
                            h3_out[b * P : b * P + rows, :], h3t[:rows, :]
                        )

    nc.compile()
    return nc


_NC_CACHE: dict = {}


def _make_in_maps(inputs, idx_cores, dstloc_cores, inv_cores):
    x = np.asarray(inputs["x"], np.float32)
    # pair-packed T1 row = [x_hi(g) | x_lo(g) | x_hi(g+1) | x_lo(g+1)] at the
    # split-permuted row for even node g (same layout the gather idx uses)
    x_hi = x.astype(BF16)
    x_lo = (x - x_hi.astype(np.float32)).astype(BF16)
    pairs = np.concatenate([x_hi, x_lo], axis=1).reshape(NPAIR, FT)
    t1 = np.empty_like(pairs)
    t1[_table_row(np.arange(0, N, 2))] = pairs
    t1 = np.ascontiguousarray(t1)

    iota = np.broadcast_to(np.arange(128, dtype=np.float32), (128, 128))
    iota = np.ascontiguousarray(iota)
    ident = np.eye(128, dtype=np.float32)

    def wrow(v):
        return np.ascontiguousarray(v.reshape(1, -1))

    common = dict(
        t1=t1,
        iota=_bf16(iota),
        ident=_bf16(ident),
        Wp=_bf16(inputs["Wp"]),
        bp=_bf16(wrow(inputs["bp"])),
        # table col order L1: [x_hi | x_lo] -> both multiply Wl1
        W1c=_bf16(np.vstack([inputs["Wl1"], inputs["Wl1"]])),
        bl1=_bf16(wrow(inputs["bl1"])),
        Wr1=_bf16(inputs["Wr1"]),
        # table col order L2: [h1 | xp] -> [Wl2[64:], Wl2[:64]]
        W2c=_bf16(np.vstack([inputs["Wl2"][64:128], inputs["Wl2"][0:64]])),
        bl2=_bf16(wrow(inputs["bl2"])),
        Wr2a=_bf16(inputs["Wr2"][0:64]),
        Wr2b=_bf16(inputs["Wr2"][64:128]),
        # s12T rows: [sum(h1); sum(xp)] -> [Wl3[64:128], Wl3[0:64]]
        W3c=_bf16(np.vstack([inputs["Wl3"][64:128], inputs["Wl3"][0:64]])),
        Wl3h2=_bf16(inputs["Wl3"][128:256]),
        bl3=_bf16(wrow(inputs["bl3"])),
        Wr3a=_bf16(inputs["Wr3"][0:64]),
        Wr3b=_bf16(inputs["Wr3"][64:128]),
        Wr3c=_bf16(inputs["Wr3"][128:256]),
    )

    in_maps = []
    for c in range(NCORES):
        xs = np.zeros((F0, SHARD_PAD), np.float32)
        xs[:, :SHARD] = x[c * SHARD : (c + 1) * SHARD].T
        in_maps.append(
            dict(
                common,
                xt=_bf16(xs),
                idx=idx_cores[c],
                dstloc=dstloc_cores[c],
                invd=inv_cores[c],
            )
        )
    return in_maps


def kernel(**inputs: np.ndarray) -> np.ndarray:
    edge_index = np.asarray(inputs["edge_index"])
    idx_cores, dstloc_cores, inv_cores, t_e_arr, t_o_arr = _preprocess(edge_index)

    ck = (t_e_arr, t_o_arr)
    if ck not in _NC_CACHE:
        _NC_CACHE[ck] = _build_nc(t_e_arr, t_o_arr)
    nc = _NC_CACHE[ck]

    in_maps = _make_in_maps(inputs, idx_cores, dstloc_cores, inv_cores)
    res = run_bass_kernel_spmd(nc, in_maps, core_ids=list(range(NCORES)))
    out = np.concatenate([res.results[c]["h3"] for c in range(NCORES)], axis=0)
    return out.astype(np.float32)


# revision 62
# speedup vs baseline: 1.2738x; 1.0121x over previous
"""Trainium2 Bass kernel for 3-layer GraphSAGE-mean (DenseGCN variant).

Strategy (8 NeuronCores, graph/data parallel):
  - Nodes sharded by range across cores (6250/core); edges binned by dst core
    and dst 128-block.
  - Gather tables are PAIR-PACKED: table row j holds nodes (2j, 2j+1), so
    int16 gather indices (pair ids < 25000) cover all 50000 nodes with no
    half-split.  Edge slots carry parity in dstloc2 (= dst%128 + 128*parity);
    a double-wide onehot [128, 256] (is_equal vs iota2 = [0..127|128..255])
    scatters the even/odd halves of each gathered row via two matmuls into
    one accumulating PSUM bank.
  - Gathers are batched: ONE dma_gather per group of G dst-blocks (SWDGE
    fixed overhead ~1us/call).
  - T1 = [x_hi|x_lo|...] bf16 (512B rows, full f32 precision via hi+lo).
    T2 = [h1|xp|...] fp8e4, T3 = [h2|...] fp8e4 (256B rows): halves the
    inter-layer AllGather size; aggregation noise from fp8 is averaged over
    ~16 neighbors.  Root-weight (Wr) paths stay bf16 via persistent
    feature-major copies, so fp8 only touches aggregated terms.
  - Wl applied AFTER aggregation per block (node-major out), 1/deg applied
    exactly in f32, root weight (Wr) + bias via separate PSUM, ReLU on ACT.
  - Sums of xp and h1 are computed once (L2) and reused at L3.
"""

import sys

sys.path.insert(0, "/opt/trn_rl_repo")

import os

import numpy as np
import ml_dtypes

import concourse.bass as bass
import concourse.bacc as bacc
import concourse.tile as tile
from concourse import mybir
from concourse.bass_utils import run_bass_kernel_spmd

BF16 = ml_dtypes.bfloat16
FP8 = ml_dtypes.float8_e4m3

N = 50000
E = 800000
NCORES = 8
SHARD = N // NCORES          # 6250
P = 128
NBLK = (SHARD + P - 1) // P  # 49
SHARD_PAD = NBLK * P         # 6272
NPAIR = N // 2               # 25000 pair rows per table

F0 = 64
O1, O2, O3 = 64, 128, 128
FT = 256                     # table row: 256 elems (two nodes' features)
LAYERS = int(os.environ.get("KERNEL_LAYERS", "3"))
G = int(os.environ.get("KERNEL_G", "2"))   # blocks per gather group
USE_FP8 = int(os.environ.get("KERNEL_FP8", "1"))
# SWDGE descriptor ring: one gather of n rows needs ~ n*8/9+1 descs; a single
# ring entry must fit the (ucode-fixed) 1024-desc ring -> calls of <= MAXCHUNK
# chunks (MAXCHUNK*128 rows).
DMA_SCRATCH = int(os.environ.get("KERNEL_DMA_SCRATCH", "16384"))
MAXCHUNK = int(os.environ.get("KERNEL_MAXCHUNK", "8"))
# The inter-layer AllGathers are split into chunk collectives over dst-block
# ranges so each chunk starts as soon as its staging sub-range is written,
# overlapping the collective with the producing layer's tail.  Table rows are
# permuted so each chunk's AllGather output is contiguous.
SPLITS = [int(x) for x in os.environ.get("KERNEL_SPLITS", "8,11,13,17").split(",")]
assert sum(SPLITS) == NBLK
_SPLIT_META = []  # (node0, node1, pairs_per_core, table_base_row)
_n0, _base = 0, 0
for _cnt in SPLITS:
    _n1 = min(_n0 + _cnt * P, SHARD)
    _pairs = (_n1 - _n0) // 2
    _SPLIT_META.append((_n0, _n1, _pairs, _base))
    _base += NCORES * _pairs
    _n0 = _n1
assert _n0 == SHARD and _base == NPAIR


def _table_row(g: np.ndarray) -> np.ndarray:
    """Global node id -> pair-table row under the split-permuted layout."""
    c = g // SHARD
    r = g % SHARD
    row = np.zeros_like(g)
    for a0, a1, pairs, base in _SPLIT_META:
        msk = (r >= a0) & (r < a1)
        row[msk] = base + c[msk] * pairs + ((r[msk] - a0) >> 1)
    return row

dt = mybir.dt
T23_DT = dt.float8e4 if USE_FP8 else dt.bfloat16
T23_NP = FP8 if USE_FP8 else BF16


def _bf16(x):
    return np.ascontiguousarray(x.astype(BF16))


def _wrap16(idx_flat: np.ndarray) -> np.ndarray:
    """[n] -> [128, n/16]: slot i at [i%16, i//16], replicated over 8 q7 cores."""
    n = idx_flat.shape[0]
    w = idx_flat.reshape(n // 16, 16).T
    return np.ascontiguousarray(np.tile(w, (8, 1)))


def _preprocess(edge_index: np.ndarray):
    """Bin edges -> per-core pair-gather idx / dstloc2 arrays and inv-degree.

    idx layout (per core) is GROUP-major: for each group g of G dst-blocks,
    the pair indices of its blocks (t_p chunks each, wrapped) are contiguous,
    so one dma_gather per group covers all its blocks.
    """
    src = edge_index[0].astype(np.int64)
    dst = edge_index[1].astype(np.int64)
    cnt = np.bincount(dst, minlength=N)
    inv = (1.0 / np.maximum(cnt, 1)).astype(np.float32)

    core = dst // SHARD
    blk = (dst % SHARD) // P
    parity = src & 1
    # slots sorted by (core, block, src-parity): chunks are parity-pure, so
    # each chunk takes a single-wide onehot and one matmul on a static half
    # of the gathered pair row.  Chunk budgets are PER BLOCK (max over cores),
    # not global, so lightly-loaded blocks aren't padded to the global worst.
    key = (core * NBLK + blk) * 2 + parity
    counts2 = np.bincount(key, minlength=NCORES * NBLK * 2).reshape(
        NCORES, NBLK, 2
    )
    t_e_arr = tuple(
        int(v) for v in np.ceil(counts2[:, :, 0].max(axis=0) / P).astype(int)
    )
    t_o_arr = tuple(
        int(v) for v in np.ceil(counts2[:, :, 1].max(axis=0) / P).astype(int)
    )
    t_b = np.array(t_e_arr) + np.array(t_o_arr)
    off_b = np.concatenate([[0], np.cumsum(t_b)])  # chunk offsets, [NBLK+1]
    tot = int(off_b[-1])

    order = np.argsort(key, kind="stable")
    skey = key[order]
    ssrc = src[order]
    sdst = dst[order]
    group_start = np.searchsorted(skey, np.arange(NCORES * NBLK * 2))
    pos = np.arange(E) - group_start[skey]

    scoreb = skey // 2
    spar = skey % 2
    b_of = scoreb % NBLK
    slot = (off_b[b_of] + spar * np.array(t_e_arr)[b_of]) * P + pos
    score = scoreb // NBLK

    ngrp = (NBLK + G - 1) // G
    idx_cores, dstloc_cores, inv_cores = [], [], []
    for c in range(NCORES):
        m = score == c
        sl = slot[m]
        sidx = np.zeros(tot * P, np.int16)
        sdl = np.full(tot * P, -1.0, np.float32)
        pair = _table_row(ssrc[m])
        assert pair.max(initial=0) < NPAIR
        sidx[sl] = pair.astype(np.int16)
        sdl[sl] = (sdst[m] % SHARD % P).astype(np.float32)

        # wrap indices group-major (ragged per block)
        segs = []
        for g in range(ngrp):
            b0, b1 = g * G, min((g + 1) * G, NBLK)
            for b in range(b0, b1):
                segs.append(_wrap16(sidx[off_b[b] * P : off_b[b + 1] * P]))
        idx_cores.append(np.ascontiguousarray(np.concatenate(segs, axis=1)))

        # dstloc [128, tot]: [p, off_b[b]+cc] = dst%128 of that chunk's slot p
        dl = sdl.reshape(tot, P).T
        dstloc_cores.append(np.ascontiguousarray(dl))

        iv = np.ones(SHARD_PAD, np.float32)
        iv[:SHARD] = inv[c * SHARD : (c + 1) * SHARD]
        inv_cores.append(np.ascontiguousarray(iv.reshape(NBLK, P).T))

    return idx_cores, dstloc_cores, inv_cores, t_e_arr, t_o_arr


def _build_nc(t_e_arr: tuple, t_o_arr: tuple):
    t_b = np.array(t_e_arr) + np.array(t_o_arr)
    off_b = np.concatenate([[0], np.cumsum(t_b)])
    tot = int(off_b[-1])
    ngrp = (NBLK + G - 1) // G
    gmax = max(
        int(off_b[min((g + 1) * G, NBLK)] - off_b[g * G]) for g in range(ngrp)
    )
    ndesc_per_call = (MAXCHUNK * P * 8) // 9 + 1
    assert ndesc_per_call <= DMA_SCRATCH // 16, (
        f"gather of {MAXCHUNK * P} rows ({ndesc_per_call} descs) exceeds the "
        f"SWDGE ring ({DMA_SCRATCH // 16} descs)"
    )
    nc = bacc.Bacc(
        "TRN2",
        target_bir_lowering=False,
        debug=False,
        dynamic_dma_scratch_size=DMA_SCRATCH,
    )

    # ---- I/O ----
    t1_in = nc.dram_tensor("t1", [NPAIR, FT], dt.bfloat16, kind="ExternalInput")
    xt_in = nc.dram_tensor("xt", [F0, SHARD_PAD], dt.bfloat16, kind="ExternalInput")
    idx_in = nc.dram_tensor(
        "idx", [128, tot * 8], dt.int16, kind="ExternalInput"
    )
    dstloc_in = nc.dram_tensor(
        "dstloc", [128, tot], dt.float32, kind="ExternalInput"
    )
    inv_in = nc.dram_tensor("invd", [128, NBLK], dt.float32, kind="ExternalInput")
    iota_in = nc.dram_tensor("iota", [128, 128], dt.bfloat16, kind="ExternalInput")
    ident_in = nc.dram_tensor("ident", [128, 128], dt.bfloat16, kind="ExternalInput")

    wp_in = nc.dram_tensor("Wp", [F0, O1], dt.bfloat16, kind="ExternalInput")
    bp_in = nc.dram_tensor("bp", [1, O1], dt.bfloat16, kind="ExternalInput")
    w1c_in = nc.dram_tensor("W1c", [128, O1], dt.bfloat16, kind="ExternalInput")
    bl1_in = nc.dram_tensor("bl1", [1, O1], dt.bfloat16, kind="ExternalInput")
    wr1_in = nc.dram_tensor("Wr1", [F0, O1], dt.bfloat16, kind="ExternalInput")
    w2c_in = nc.dram_tensor("W2c", [128, O2], dt.bfloat16, kind="ExternalInput")
    bl2_in = nc.dram_tensor("bl2", [1, O2], dt.bfloat16, kind="ExternalInput")
    wr2a_in = nc.dram_tensor("Wr2a", [64, O2], dt.bfloat16, kind="ExternalInput")
    wr2b_in = nc.dram_tensor("Wr2b", [64, O2], dt.bfloat16, kind="ExternalInput")
    w3c_in = nc.dram_tensor("W3c", [128, O3], dt.bfloat16, kind="ExternalInput")
    wl3h2_in = nc.dram_tensor("Wl3h2", [128, O3], dt.bfloat16, kind="ExternalInput")
    bl3_in = nc.dram_tensor("bl3", [1, O3], dt.bfloat16, kind="ExternalInput")
    wr3a_in = nc.dram_tensor("Wr3a", [64, O3], dt.bfloat16, kind="ExternalInput")
    wr3b_in = nc.dram_tensor("Wr3b", [64, O3], dt.bfloat16, kind="ExternalInput")
    wr3c_in = nc.dram_tensor("Wr3c", [128, O3], dt.bfloat16, kind="ExternalInput")

    h3_out = nc.dram_tensor("h3", [SHARD, O3], dt.float32, kind="ExternalOutput")

    with tile.TileContext(nc) as tc:
        with (
            tc.tile_pool(name="cons", bufs=1) as cons,
            tc.tile_pool(name="sbuf", bufs=2) as sb,
            tc.tile_pool(name="psum", bufs=2, space="PSUM") as ps,
            tc.tile_pool(name="dram", bufs=1, space="DRAM") as dr,
        ):
            # ---- constants -> SBUF ----
            iota_t = cons.tile([128, 128], dt.bfloat16)
            nc.sync.dma_start(iota_t[:], iota_in[:])
            ident_t = cons.tile([128, 128], dt.bfloat16)
            nc.sync.dma_start(ident_t[:], ident_in[:])
            idx_t = cons.tile([128, tot * 8], dt.int16)
            nc.sync.dma_start(idx_t[:], idx_in[:])
            dstloc_t = cons.tile([128, tot], dt.float32)
            nc.sync.dma_start(dstloc_t[:], dstloc_in[:])
            inv_t = cons.tile([128, NBLK], dt.float32)
            nc.sync.dma_start(inv_t[:], inv_in[:])
            xt_t = cons.tile([F0, SHARD_PAD], dt.bfloat16)
            nc.sync.dma_start(xt_t[:], xt_in[:])
            ones_t = cons.tile([1, 128], dt.bfloat16)
            nc.gpsimd.memset(ones_t[:], 1.0)

            def load_w(name, src, shape):
                t = cons.tile(list(shape), dt.bfloat16, name=name)
                nc.sync.dma_start(t[:], src[:])
                return t

            wp_t = load_w("wp_t", wp_in, (F0, O1))
            bp_t = load_w("bp_t", bp_in, (1, O1))
            w1c_t = load_w("w1c_t", w1c_in, (128, O1))
            bl1_t = load_w("bl1_t", bl1_in, (1, O1))
            wr1_t = load_w("wr1_t", wr1_in, (F0, O1))
            w2c_t = load_w("w2c_t", w2c_in, (128, O2))
            bl2_t = load_w("bl2_t", bl2_in, (1, O2))
            wr2a_t = load_w("wr2a_t", wr2a_in, (64, O2))
            wr2b_t = load_w("wr2b_t", wr2b_in, (64, O2))
            w3c_t = load_w("w3c_t", w3c_in, (128, O3))
            wl3h2_t = load_w("wl3h2_t", wl3h2_in, (128, O3))
            bl3_t = load_w("bl3_t", bl3_in, (1, O3))
            wr3a_t = load_w("wr3a_t", wr3a_in, (64, O3))
            wr3b_t = load_w("wr3b_t", wr3b_in, (64, O3))
            wr3c_t = load_w("wr3c_t", wr3c_in, (128, O3))

            # ---- persistent feature-major activations (local shard) ----
            xpT = cons.tile([O1, SHARD_PAD], dt.bfloat16)
            h1T = cons.tile([O1, SHARD_PAD], dt.bfloat16)
            h2T = cons.tile([O2, SHARD_PAD], dt.bfloat16)
            s12T = cons.tile([128, SHARD_PAD], dt.bfloat16)  # saved [sum(h1); sum(xp)]
            xp_sb = cons.tile([128, NBLK, 64], dt.bfloat16)  # xp staging (bf16)

            # ---- DRAM tables (pair-packed rows, split-permuted layout) ----
            t2_shards = [
                dr.tile([m[2], FT], T23_DT, name=f"t2_shard{s}")
                for s, m in enumerate(_SPLIT_META)
            ]
            t2_full = dr.tile([NPAIR, FT], T23_DT)
            t3_shards = [
                dr.tile([m[2], FT], T23_DT, name=f"t3_shard{s}")
                for s, m in enumerate(_SPLIT_META)
            ]
            t3_full = dr.tile([NPAIR, FT], T23_DT)

            def stage_block(shards, b, st, rows):
                """Write block b's [rows,128] staging to its split shard."""
                s = 0
                while b * P >= _SPLIT_META[s][1]:
                    s += 1
                off = b * 64 - _SPLIT_META[s][0] // 2
                nc.sync.dma_start(
                    shards[s][off : off + rows // 2, :], st[:rows, :]
                )

            # group index after which split s's staging is complete
            _emit_after = {}
            for s, (a0, a1, pairs, base) in enumerate(_SPLIT_META):
                last_blk = (a1 + P - 1) // P - 1
                _emit_after.setdefault(last_blk // G, []).append(s)

            def allgather_chunk(shards, full, s):
                a0, a1, pairs, base = _SPLIT_META[s]
                nc.gpsimd.collective_compute(
                    "AllGather",
                    mybir.AluOpType.bypass,
                    replica_groups=[list(range(NCORES))],
                    ins=[shards[s][:]],
                    outs=[full[base : base + NCORES * pairs, :]],
                )

            def transpose_to(dst_col0, src_nm, rows, name):
                """src_nm [128, rows] bf16 node-major -> dst slice [rows, 128]."""
                pt = ps.tile([rows, 128], dt.bfloat16, space="PSUM", tag="pt")
                nc.tensor.transpose(out=pt[:], in_=src_nm, identity=ident_t[:])
                nc.vector.tensor_copy(out=dst_col0, in_=pt[:])

            def gather_group(g, table, ygdt, ygtag):
                """One batched pair-gather for group g's blocks -> yg tile."""
                b0 = g * G
                nb = min(G, NBLK - b0)
                goff = int(off_b[b0])
                gch = int(off_b[b0 + nb]) - goff
                yg = sb.tile([128, gmax, FT], ygdt, tag=ygtag, bufs=2)
                ibase = goff * 8
                for s0 in range(0, gch, MAXCHUNK):
                    s1 = min(s0 + MAXCHUNK, gch)
                    n_idx = (s1 - s0) * P
                    nc.gpsimd.dma_gather(
                        yg[:, s0:s1, :],
                        table,
                        idx_t[:, ibase + s0 * 8 : ibase + s1 * 8],
                        n_idx,
                        n_idx,
                        FT,
                    )
                return yg, nb

            def agg_block(yg, nb, bl, b):
                """Onehot matmuls for block b (local bl) -> psum [128f, 128dst].

                Chunks cc < t_e_arr[b] hold even-src slots (pair row half
                0:128), the rest odd-src slots (half 128:256)."""
                pagg = ps.tile([128, 128], dt.float32, space="PSUM", tag="pagg")
                boff = int(off_b[b])
                goff = int(off_b[b - bl])
                tp_b = int(t_b[b])
                for cc in range(tp_b):
                    pos = boff - goff + cc
                    half = slice(0, 128) if cc < t_e_arr[b] else slice(128, 256)
                    oh = sb.tile([128, 128], dt.bfloat16, tag="oh", bufs=16)
                    nc.vector.tensor_scalar(
                        out=oh[:],
                        in0=iota_t[:],
                        scalar1=dstloc_t[:, boff + cc : boff + cc + 1],
                        scalar2=None,
                        op0=mybir.AluOpType.is_equal,
                    )
                    nc.tensor.matmul(
                        out=pagg[:],
                        lhsT=yg[:, pos, half],
                        rhs=oh[:],
                        start=(cc == 0),
                        stop=(cc == tp_b - 1),
                    )
                return pagg

            def epilogue(b, pmean, pxr, out_ap):
                """out = relu(pmean * inv + pxr); writes to out_ap."""
                ocols = pmean.shape[-1]
                tmul = sb.tile([128, ocols], dt.float32, tag="tmul")
                nc.vector.tensor_scalar(
                    out=tmul[:],
                    in0=pmean[:],
                    scalar1=inv_t[:, b : b + 1],
                    scalar2=None,
                    op0=mybir.AluOpType.mult,
                )
                tadd = sb.tile([128, ocols], dt.float32, tag="tadd")
                nc.vector.tensor_tensor(
                    out=tadd[:], in0=tmul[:], in1=pxr[:], op=mybir.AluOpType.add
                )
                nc.scalar.activation(
                    out=out_ap, in_=tadd[:], func=mybir.ActivationFunctionType.Relu
                )

            # ================= Phase 1: xp per block =================
            for b in range(NBLK):
                xtb = xt_t[:, b * P : (b + 1) * P]
                pxp = ps.tile([128, O1], dt.float32, space="PSUM", tag="pmean")
                nc.tensor.matmul(
                    out=pxp[:], lhsT=xtb, rhs=wp_t[:], start=True, stop=False
                )
                nc.tensor.matmul(
                    out=pxp[:], lhsT=ones_t[:], rhs=bp_t[:], start=False, stop=True
                )
                nc.scalar.activation(
                    out=xp_sb[:, b, :],
                    in_=pxp[:],
                    func=mybir.ActivationFunctionType.Relu,
                )
                transpose_to(xpT[:, b * P : (b + 1) * P], xp_sb[:, b, :], O1, "xp")

            # ================= Phase 2: layer 1 =================
            for g in range(ngrp):
                yg, nb = gather_group(g, t1_in[:], dt.bfloat16, "yg1")
                for bl in range(nb):
                    b = g * G + bl
                    pagg = agg_block(yg, nb, bl, b)
                    # [sum(x_hi); sum(x_lo)] -> one K=128 matmul with [Wl1; Wl1]
                    sxT = sb.tile([128, 128], dt.bfloat16, tag="sxT")
                    nc.vector.tensor_copy(out=sxT[:], in_=pagg[:])
                    pmean = ps.tile([128, O1], dt.float32, space="PSUM", tag="pmean")
                    nc.tensor.matmul(
                        out=pmean[:], lhsT=sxT[:], rhs=w1c_t[:], start=True, stop=True
                    )
                    pxr = ps.tile([128, O1], dt.float32, space="PSUM", tag="pxr")
                    xtb = xt_t[:, b * P : (b + 1) * P]
                    nc.tensor.matmul(
                        out=pxr[:], lhsT=xtb, rhs=wr1_t[:], start=True, stop=False
                    )
                    nc.tensor.matmul(
                        out=pxr[:], lhsT=ones_t[:], rhs=bl1_t[:], start=False, stop=True
                    )
                    # h1 (bf16) -> rotating staging; then [h1|xp] -> T23 dtype tile
                    h1b = sb.tile([128, 64], dt.bfloat16, tag="h1b", bufs=4)
                    epilogue(b, pmean, pxr, h1b[:])
                    transpose_to(h1T[:, b * P : (b + 1) * P], h1b[:], O1, "h1")
                    st2 = sb.tile([128, 128], T23_DT, tag="st2", bufs=4)
                    nc.vector.tensor_copy(out=st2[:, 0:64], in_=h1b[:])
                    nc.vector.tensor_copy(out=st2[:, 64:128], in_=xp_sb[:, b, :])
                    # [128 nodes, 128B] bytes == [64 pair rows, 256B]
                    rows = min(P, SHARD - b * P)
                    stage_block(t2_shards, b, st2, rows)
                if LAYERS >= 2:
                    for s_ in _emit_after.get(g, []):
                        allgather_chunk(t2_shards, t2_full, s_)

            if LAYERS == 1:
                for b in range(NBLK):
                    rows = min(P, SHARD - b * P)
                    nc.gpsimd.dma_start(
                        out=h3_out[b * P : b * P + rows, 0:64],
                        in_=h1T[:, b * P : b * P + rows],
                    )

            if LAYERS >= 2:
                # ================= Phase 3: layer 2 =================
                for g in range(ngrp):
                    yg, nb = gather_group(g, t2_full[:], T23_DT, "yg")
                    for bl in range(nb):
                        b = g * G + bl
                        pagg = agg_block(yg, nb, bl, b)
                        # rows 0:64 = sum(h1), 64:128 = sum(xp); save whole block
                        nc.vector.tensor_copy(
                            out=s12T[:, b * P : (b + 1) * P], in_=pagg[:]
                        )
                        pmean = ps.tile([128, O2], dt.float32, space="PSUM", tag="pmean")
                        nc.tensor.matmul(
                            out=pmean[:],
                            lhsT=s12T[:, b * P : (b + 1) * P],
                            rhs=w2c_t[:],
                            start=True,
                            stop=True,
                        )
                        pxr = ps.tile([128, O2], dt.float32, space="PSUM", tag="pxr")
                        nc.tensor.matmul(
                            out=pxr[:],
                            lhsT=xpT[:, b * P : (b + 1) * P],
                            rhs=wr2a_t[:],
                            start=True,
                            stop=False,
                        )
                        nc.tensor.matmul(
                            out=pxr[:],
                            lhsT=h1T[:, b * P : (b + 1) * P],
                            rhs=wr2b_t[:],
                            start=False,
                            stop=False,
                        )
                        nc.tensor.matmul(
                            out=pxr[:], lhsT=ones_t[:], rhs=bl2_t[:],
                            start=False, stop=True,
                        )
                        h2b = sb.tile([128, 128], dt.bfloat16, tag="h2b", bufs=4)
                        epilogue(b, pmean, pxr, h2b[:])
                        transpose_to(h2T[:, b * P : (b + 1) * P], h2b[:], O2, "h2")
                        st3 = sb.tile([128, 128], T23_DT, tag="st2", bufs=4)
                        nc.vector.tensor_copy(out=st3[:], in_=h2b[:])
                        rows = min(P, SHARD - b * P)
                        stage_block(t3_shards, b, st3, rows)
                    if LAYERS >= 3:
                        for s_ in _emit_after.get(g, []):
                            allgather_chunk(t3_shards, t3_full, s_)

            if LAYERS == 2:
                for b in range(NBLK):
                    rows = min(P, SHARD - b * P)
                    nc.gpsimd.dma_start(
                        out=h3_out[b * P : b * P + rows, :],
                        in_=h2T[:, b * P : b * P + rows],
                    )

            if LAYERS >= 3:
                # ================= Phase 4: layer 3 =================
                for g in range(ngrp):
                    yg, nb = gather_group(g, t3_full[:], T23_DT, "yg")
                    for bl in range(nb):
                        b = g * G + bl
                        pagg = agg_block(yg, nb, bl, b)
                        sh2T = sb.tile([128, 128], dt.bfloat16, tag="sh2T")
                        nc.vector.tensor_copy(out=sh2T[:], in_=pagg[:])
                        pmean = ps.tile([128, O3], dt.float32, space="PSUM", tag="pmean")
                        nc.tensor.matmul(
                            out=pmean[:],
                            lhsT=s12T[:, b * P : (b + 1) * P],
                            rhs=w3c_t[:],
                            start=True,
                            stop=False,
                        )
                        nc.tensor.matmul(
                            out=pmean[:],
                            lhsT=sh2T[:],
                            rhs=wl3h2_t[:],
                            start=False,
                            stop=True,
                        )
                        pxr = ps.tile([128, O3], dt.float32, space="PSUM", tag="pxr")
                        nc.tensor.matmul(
                            out=pxr[:],
                            lhsT=xpT[:, b * P : (b + 1) * P],
                            rhs=wr3a_t[:],
                            start=True,
                            stop=False,
                        )
                        nc.tensor.matmul(
                            out=pxr[:],
                            lhsT=h1T[:, b * P : (b + 1) * P],
                            rhs=wr3b_t[:],
                            start=False,
                            stop=False,
                        )
                        nc.tensor.matmul(
                            out=pxr[:],
                            lhsT=h2T[:, b * P : (b + 1) * P],
                            rhs=wr3c_t[:],
                            start=False,
                            stop=False,
                        )
                        nc.tensor.matmul(
                            out=pxr[:], lhsT=ones_t[:], rhs=bl3_t[:],
                            start=False, stop=True,
                        )
                        h3t = sb.tile([128, O3], dt.float32, tag="h3t")
                        epilogue(b, pmean, pxr, h3t[:])
                        rows = min(P, SHARD - b * P)
                        nc.sync.dma_start(
                            h3_out[b * P : b * P + rows, :], h3t[:rows, :]
                        )

    nc.compile()
    return nc


_NC_CACHE: dict = {}


def _make_in_maps(inputs, idx_cores, dstloc_cores, inv_cores):
    x = np.asarray(inputs["x"], np.float32)
    # pair-packed T1 row = [x_hi(g) | x_lo(g) | x_hi(g+1) | x_lo(g+1)] at the
    # split-permuted row for even node g (same layout the gather idx uses)
    x_hi = x.astype(BF16)
    x_lo = (x - x_hi.astype(np.float32)).astype(BF16)
    pairs = np.concatenate([x_hi, x_lo], axis=1).reshape(NPAIR, FT)
    t1 = np.empty_like(pairs)
    t1[_table_row(np.arange(0, N, 2))] = pairs
    t1 = np.ascontiguousarray(t1)

    iota = np.broadcast_to(np.arange(128, dtype=np.float32), (128, 128))
    iota = np.ascontiguousarray(iota)
    ident = np.eye(128, dtype=np.float32)

    def wrow(v):
        return np.ascontiguousarray(v.reshape(1, -1))

    common = dict(
        t1=t1,
        iota=_bf16(iota),
        ident=_bf16(ident),
        Wp=_bf16(inputs["Wp"]),
        bp=_bf16(wrow(inputs["bp"])),
        # table col order L1: [x_hi | x_lo] -> both multiply Wl1
        W1c=_bf16(np.vstack([inputs["Wl1"], inputs["Wl1"]])),
        bl1=_bf16(wrow(inputs["bl1"])),
        Wr1=_bf16(inputs["Wr1"]),
        # table col order L2: [h1 | xp] -> [Wl2[64:], Wl2[:64]]
        W2c=_bf16(np.vstack([inputs["Wl2"][64:128], inputs["Wl2"][0:64]])),
        bl2=_bf16(wrow(inputs["bl2"])),
        Wr2a=_bf16(inputs["Wr2"][0:64]),
        Wr2b=_bf16(inputs["Wr2"][64:128]),
        # s12T rows: [sum(h1); sum(xp)] -> [Wl3[64:128], Wl3[0:64]]
        W3c=_bf16(np.vstack([inputs["Wl3"][64:128], inputs["Wl3"][0:64]])),
        Wl3h2=_bf16(inputs["Wl3"][128:256]),
        bl3=_bf16(wrow(inputs["bl3"])),
        Wr3a=_bf16(inputs["Wr3"][0:64]),
        Wr3b=_bf16(inputs["Wr3"][64:128]),
        Wr3c=_bf16(inputs["Wr3"][128:256]),
    )

    in_maps = []
    for c in range(NCORES):
        xs = np.zeros((F0, SHARD_PAD), np.float32)
        xs[:, :SHARD] = x[c * SHARD : (c + 1) * SHARD].T
        in_maps.append(
            dict(
                common,
                xt=_bf16(xs),
                idx=idx_cores[c],
                dstloc=dstloc_cores[c],
                invd=inv_cores[c],
            )
        )
    return in_maps


def kernel(**inputs: np.ndarray) -> np.ndarray:
    edge_index = np.asarray(inputs["edge_index"])
    idx_cores, dstloc_cores, inv_cores, t_e_arr, t_o_arr = _preprocess(edge_index)

    ck = (t_e_arr, t_o_arr)
    if ck not in _NC_CACHE:
        _NC_CACHE[ck] = _build_nc(t_e_arr, t_o_arr)
    nc = _NC_CACHE[ck]

    in_maps = _make_in_maps(inputs, idx_cores, dstloc_cores, inv_cores)
    res = run_bass_kernel_spmd(nc, in_maps, core_ids=list(range(NCORES)))
    out = np.concatenate([res.results[c]["h3"] for c in range(NCORES)], axis=0)
    return out.astype(np.float32)
